# revision 4
# baseline (speedup 1.0000x reference)
"""Dense full attention (RoPE, causal) Trainium2 Bass kernel.

Problem: B=4, S=2048, D=2048, H=16 heads x HD=128.
  q = (x @ wq), k = (x @ wk), v = (x @ wv); RoPE on q, k;
  out = softmax(causal(q k^T / sqrt(HD))) v @ wo

Sharding over 8 cores: core c handles batch b = c // 2 and head-group
g = c % 2 (8 heads, 1024 of the 2048 qkv columns / wo rows). Each core
returns a partial [S, D] output; the host sums the two head-group
partials per batch.

Per-core device schedule (one SPMD program, no collectives):
  Phase A: QKV projections off a resident x^T (host pre-transposed).
    Q^T, K^T computed head-wise [128, S] with RoPE fused at PSUM
    eviction; V computed in natural [S, c] layout. All spilled to DRAM.
  Phase B: per head, S^T blocks via matmul(lhsT=K^T tile, rhs=Q^T
    block) -> exp on ScalarE (no max subtraction; logits are O(10) so
    fp32 exp is safe) -> causal mask by multiplicative mask tiles ->
    PV matmul (lhsT=V tile, rhs=exp block) accumulating O^T [128, S]
    in PSUM. Softmax denominators l via ones-vector matmul over a
    DVE-accumulated exp sum; O^T multiplied by broadcast 1/l on
    eviction.
  Phase C: out = O^T.T @ wo with wo resident, 8 K-subtiles per block.

Matmul compute dtype is float32r (TF32-like: 1 cycle/row at N>=256 vs
4 for fp32). The walrus verifier requires every matmul input to be
produced with dtype float32r, so matmul-feeding DRAM tensors and SBUF
tiles are declared float32r (same 4-byte layout; np.float32 on host).
PSUM accumulation stays fp32.
"""

import os
import sys

import numpy as np

for _p in ("/opt/trn_rl_repo", os.path.expanduser("~/.axon_site/_ro/trn_rl_repo")):
    if os.path.isdir(_p) and _p not in sys.path:
        sys.path.insert(0, _p)

import concourse.bass as bass
import concourse.mybir as mybir
import concourse.tile as tile
from concourse import bacc

B, S, D = 4, 2048, 2048
H, HD = 16, 128
P = 128
NH = 8          # heads per core
CC = NH * HD    # qkv columns per core (1024)
DS = D // P     # contraction subtiles (16)
ST = S // 512   # 512-wide query blocks (4)
SB = S // P     # 128-row blocks (16)
KO = CC // P    # out-proj contraction subtiles (8)

F32 = mybir.dt.float32
F32R = mybir.dt.float32r
SOFTMAX_SCALE = 1.0 / float(np.sqrt(HD))


def build_nc():
    nc = bacc.Bacc()

    xT_d = nc.dram_tensor("xT", [D, S], F32R, kind="ExternalInput")
    wq_d = nc.dram_tensor("wq", [D, CC], F32R, kind="ExternalInput")
    wk_d = nc.dram_tensor("wk", [D, CC], F32R, kind="ExternalInput")
    wv_d = nc.dram_tensor("wv", [D, CC], F32R, kind="ExternalInput")
    wo_d = nc.dram_tensor("wo", [CC, D], F32R, kind="ExternalInput")
    cosT_d = nc.dram_tensor("cosT", [P, S], F32, kind="ExternalInput")
    sinT_d = nc.dram_tensor("sinT", [P, S], F32, kind="ExternalInput")
    masks_d = nc.dram_tensor("masks", [P, 4 * 512], F32R, kind="ExternalInput")
    onesc_d = nc.dram_tensor("ones_col", [P, 1], F32R, kind="ExternalInput")
    onesr_d = nc.dram_tensor("ones_row", [1, P], F32R, kind="ExternalInput")

    qt_d = nc.dram_tensor("qt", [CC, S], F32R)
    kt_d = nc.dram_tensor("kt", [CC, S], F32R)
    v_d = nc.dram_tensor("v", [S, CC], F32R)
    ot_d = nc.dram_tensor("ot", [CC, S], F32R)

    out_d = nc.dram_tensor("out", [S, D], F32, kind="ExternalOutput")

    lowp = nc.allow_low_precision(reason="float32r rounding of matmul inputs")
    lowp.__enter__()
    with tile.TileContext(nc) as tc:
        with tc.tile_pool(name="xt", bufs=1) as xt_pool:
            XT = xt_pool.tile([P, DS, S], F32R)
            nc.sync.dma_start(XT, xT_d[:].rearrange("(ko p) s -> p ko s", p=P))

            # ---- Phase A1: Q^T, K^T per head with fused RoPE ----
            with (
                tc.tile_pool(name="aqk", bufs=1) as pa,
                tc.tile_pool(name="psA", bufs=1, space="PSUM") as psA,
            ):
                cosT = pa.tile([P, S], F32)
                sinT = pa.tile([P, S], F32)
                nc.sync.dma_start(cosT, cosT_d[:])
                nc.sync.dma_start(sinT, sinT_d[:])
                for h in range(NH):
                    for w_d_, dst_d in ((wq_d, qt_d), (wk_d, kt_d)):
                        wt = pa.tile([P, DS, P], F32R, tag="w", bufs=2)
                        nc.sync.dma_start(
                            wt,
                            w_d_[:, h * P : (h + 1) * P].rearrange(
                                "(ko p) c -> p ko c", p=P
                            ),
                        )
                        ps = [
                            psA.tile([P, 512], F32, tag="proj", bufs=8, name=f"proj{t}")
                            for t in range(ST)
                        ]
                        for ks in range(DS):
                            for t in range(ST):
                                nc.tensor.matmul(
                                    ps[t],
                                    wt[:, ks],
                                    XT[:, ks, t * 512 : (t + 1) * 512],
                                    start=(ks == 0),
                                    stop=(ks == DS - 1),
                                )
                        for t in range(ST):
                            sl = slice(t * 512, (t + 1) * 512)
                            rot = pa.tile([P, 512], F32, tag="rot", bufs=3)
                            ev = pa.tile([P, 512], F32R, tag="ev", bufs=3)
                            nc.scalar.mul(rot[0:64], ps[t][64:128], -1.0)
                            nc.scalar.copy(rot[64:128], ps[t][0:64])
                            nc.vector.tensor_mul(rot, rot, sinT[:, sl])
                            nc.vector.tensor_mul(ev, ps[t], cosT[:, sl])
                            nc.vector.tensor_add(ev, ev, rot)
                            nc.sync.dma_start(dst_d[h * P : (h + 1) * P, sl], ev)

            # ---- Phase A2: V in natural [S, c] layout ----
            with (
                tc.tile_pool(name="av", bufs=1) as pv,
                tc.tile_pool(name="psV", bufs=1, space="PSUM") as psV,
            ):
                for nh in range(2):
                    wv_t = pv.tile([P, DS, 512], F32R, tag="wv", bufs=1)
                    nc.sync.dma_start(
                        wv_t,
                        wv_d[:, nh * 512 : (nh + 1) * 512].rearrange(
                            "(ko p) c -> p ko c", p=P
                        ),
                    )
                    for sb in range(SB):
                        ps = psV.tile([P, 512], F32, tag="vp", bufs=4)
                        for ks in range(DS):
                            nc.tensor.matmul(
                                ps,
                                XT[:, ks, sb * P : (sb + 1) * P],
                                wv_t[:, ks],
                                start=(ks == 0),
                                stop=(ks == DS - 1),
                            )
                        vev = pv.tile([P, 512], F32R, tag="vev", bufs=4)
                        nc.vector.tensor_copy(vev, ps)
                        nc.sync.dma_start(
                            v_d[sb * P : (sb + 1) * P, nh * 512 : (nh + 1) * 512], vev
                        )

        # ---- Phase B: attention per head ----
        with (
            tc.tile_pool(name="attn", bufs=1) as pb,
            tc.tile_pool(name="psB", bufs=1, space="PSUM") as psB,
        ):
            msk = pb.tile([P, 4, 512], F32R)
            nc.sync.dma_start(msk, masks_d[:].rearrange("p (m f) -> p m f", m=4))
            ones_col = pb.tile([P, 1], F32R)
            ones_row = pb.tile([1, P], F32R)
            nc.sync.dma_start(ones_col, onesc_d[:])
            nc.sync.dma_start(ones_row, onesr_d[:])
            for h in range(NH):
                hsl = slice(h * P, (h + 1) * P)
                QT = pb.tile([P, S], F32R, tag="qt", bufs=2)
                KT = pb.tile([P, S], F32R, tag="kt", bufs=2)
                Vt = pb.tile([P, SB, P], F32R, tag="v", bufs=2)
                nc.sync.dma_start(QT, qt_d[hsl, :])
                nc.sync.dma_start(KT, kt_d[hsl, :])
                nc.sync.dma_start(
                    Vt, v_d[:].rearrange("(jo p) c -> p jo c", p=P)[:, :, hsl]
                )
                lacc = pb.tile([P, ST, 512], F32, tag="lacc", bufs=2)
                o_ps = [
                    psB.tile([P, 512], F32, tag=f"o{t}", bufs=1, name=f"o{t}")
                    for t in range(ST)
                ]
                for jj in range(SB):
                    t0 = jj // 4
                    for t in range(t0, ST):
                        s_ps = psB.tile([P, 512], F32, tag="s", bufs=2)
                        nc.tensor.matmul(
                            s_ps,
                            KT[:, jj * P : (jj + 1) * P],
                            QT[:, t * 512 : (t + 1) * 512],
                            start=True,
                            stop=True,
                        )
                        ex = pb.tile([P, 512], F32R, tag="ex", bufs=4)
                        nc.scalar.activation(
                            ex,
                            s_ps,
                            mybir.ActivationFunctionType.Exp,
                            scale=SOFTMAX_SCALE,
                        )
                        if t == t0:
                            nc.vector.tensor_mul(ex, ex, msk[:, jj % 4])
                        if jj == 0:
                            nc.vector.tensor_copy(lacc[:, t], ex)
                        else:
                            nc.vector.tensor_add(lacc[:, t], lacc[:, t], ex)
                        nc.tensor.matmul(
                            o_ps[t],
                            Vt[:, jj],
                            ex,
                            start=(jj == 0),
                            stop=(jj == 4 * t + 3),
                        )
                for t in range(ST):
                    lacc_r = pb.tile([P, 512], F32R, tag="laccr", bufs=2)
                    nc.vector.tensor_copy(lacc_r, lacc[:, t])
                    l_ps = psB.tile([1, 512], F32, tag="l", bufs=1)
                    nc.tensor.matmul(l_ps, ones_col, lacc_r, start=True, stop=True)
                    linv = pb.tile([1, 512], F32R, tag="linv", bufs=2)
                    nc.vector.reciprocal(linv, l_ps)
                    li_ps = psB.tile([P, 512], F32, tag="libc", bufs=1)
                    nc.tensor.matmul(li_ps, ones_row, linv, start=True, stop=True)
                    li_sb = pb.tile([P, 512], F32, tag="lisb", bufs=2)
                    nc.vector.tensor_copy(li_sb, li_ps)
                    oev = pb.tile([P, 512], F32R, tag="oev", bufs=3)
                    nc.vector.tensor_mul(oev, o_ps[t], li_sb)
                    nc.sync.dma_start(ot_d[hsl, t * 512 : (t + 1) * 512], oev)

        # ---- Phase C: output projection ----
        with (
            tc.tile_pool(name="oproj", bufs=1) as pc,
            tc.tile_pool(name="psC", bufs=1, space="PSUM") as psC,
        ):
            WO = pc.tile([P, KO, D], F32R)
            nc.sync.dma_start(WO, wo_d[:].rearrange("(ko p) n -> p ko n", p=P))
            for sb in range(SB):
                ott = pc.tile([P, KO, P], F32R, tag="ot", bufs=3)
                nc.sync.dma_start(
                    ott,
                    ot_d[:].rearrange("(ko p) s -> p ko s", p=P)[
                        :, :, sb * P : (sb + 1) * P
                    ],
                )
                for t in range(ST):
                    ps = psC.tile([P, 512], F32, tag="op", bufs=4)
                    for ko in range(KO):
                        nc.tensor.matmul(
                            ps,
                            ott[:, ko],
                            WO[:, ko, t * 512 : (t + 1) * 512],
                            start=(ko == 0),
                            stop=(ko == KO - 1),
                        )
                    cev = pc.tile([P, 512], F32, tag="cev", bufs=3)
                    nc.vector.tensor_copy(cev, ps)
                    nc.sync.dma_start(
                        out_d[sb * P : (sb + 1) * P, t * 512 : (t + 1) * 512], cev
                    )

    lowp.__exit__(None, None, None)
    nc.finalize()
    return nc


_NC = None


def _get_nc():
    global _NC
    if _NC is None:
        _NC = build_nc()
    return _NC


def make_in_maps(x, wq, wk, wv, wo, cos, sin):
    """Build the 8 per-core input maps (host-side sharding/layout)."""
    cosT = np.ascontiguousarray(
        np.concatenate([cos.T, cos.T], axis=0), dtype=np.float32
    )  # [128, S]
    sinT = np.ascontiguousarray(
        np.concatenate([sin.T, sin.T], axis=0), dtype=np.float32
    )
    # causal masks for the 4 diagonal j-positions inside a 512-wide query
    # block: keep key (128*m + p) <= query f
    pidx = np.arange(P)[:, None]
    fidx = np.arange(512)[None, :]
    masks = np.concatenate(
        [(fidx >= 128 * m + pidx).astype(np.float32) for m in range(4)], axis=1
    )  # [128, 2048]
    ones_col = np.ones((P, 1), dtype=np.float32)
    ones_row = np.ones((1, P), dtype=np.float32)

    in_maps = []
    for c in range(8):
        b, g = divmod(c, 2)
        csl = slice(g * CC, (g + 1) * CC)
        in_maps.append(
            {
                "xT": np.ascontiguousarray(x[b].T),
                "wq": np.ascontiguousarray(wq[:, csl]),
                "wk": np.ascontiguousarray(wk[:, csl]),
                "wv": np.ascontiguousarray(wv[:, csl]),
                "wo": np.ascontiguousarray(wo[csl, :]),
                "cosT": cosT,
                "sinT": sinT,
                "masks": masks,
                "ones_col": ones_col,
                "ones_row": ones_row,
            }
        )
    return in_maps


def combine(results):
    out = np.empty((B, S, D), dtype=np.float32)
    for b in range(B):
        out[b] = results[2 * b]["out"] + results[2 * b + 1]["out"]
    return out


def kernel(x, wq, wk, wv, wo, cos, sin):
    from concourse.bass_utils import run_bass_kernel_spmd

    nc = _get_nc()
    in_maps = make_in_maps(
        np.asarray(x, dtype=np.float32),
        np.asarray(wq, dtype=np.float32),
        np.asarray(wk, dtype=np.float32),
        np.asarray(wv, dtype=np.float32),
        np.asarray(wo, dtype=np.float32),
        np.asarray(cos, dtype=np.float32),
        np.asarray(sin, dtype=np.float32),
    )
    res = run_bass_kernel_spmd(nc, in_maps, core_ids=list(range(8)))
    return combine(res.results)


# revision 8
# speedup vs baseline: 1.2342x; 1.2342x over previous
"""Dense full attention (RoPE, causal) Trainium2 Bass kernel.

Problem: B=4, S=2048, D=2048, H=16 heads x HD=128.
  q = (x @ wq), k = (x @ wk), v = (x @ wv); RoPE on q, k;
  out = softmax(causal(q k^T / sqrt(HD))) v @ wo

Sharding over 8 cores: core c handles batch b = c // 2 and head-group
g = c % 2 (8 heads, 1024 of the 2048 qkv columns / wo rows). Each core
returns a partial [S, D] output; the host sums the two head-group
partials per batch.

Per-core device schedule (one SPMD program, no collectives):
  Phase A: QKV projections off a resident x^T (host pre-transposed).
    Q^T, K^T computed head-wise [128, S] with RoPE fused at PSUM
    eviction; V computed in natural [S, c] layout. All spilled to DRAM.
  Phase B: per head, S^T blocks via matmul(lhsT=K^T tile, rhs=Q^T
    block) -> exp on ScalarE (no max subtraction; logits are O(10) so
    fp32 exp is safe) -> causal mask by multiplicative mask tiles ->
    PV matmul (lhsT=V tile, rhs=exp block) accumulating O^T [128, S]
    in PSUM. Softmax denominators l via ones-vector matmul over a
    DVE-accumulated exp sum; O^T multiplied by broadcast 1/l on
    eviction.
  Phase C: out = O^T.T @ wo with wo resident, 8 K-subtiles per block.

Matmul compute dtype is float32r (TF32-like: 1 cycle/row at N>=256 vs
4 for fp32). The walrus verifier requires every matmul input to be
produced with dtype float32r, so matmul-feeding DRAM tensors and SBUF
tiles are declared float32r (same 4-byte layout; np.float32 on host).
PSUM accumulation stays fp32.
"""

import os
import sys

import numpy as np

for _p in ("/opt/trn_rl_repo", os.path.expanduser("~/.axon_site/_ro/trn_rl_repo")):
    if os.path.isdir(_p) and _p not in sys.path:
        sys.path.insert(0, _p)

import concourse.bass as bass
import concourse.mybir as mybir
import concourse.tile as tile
from concourse import bacc

B, S, D = 4, 2048, 2048
H, HD = 16, 128
P = 128
NH = 8          # heads per core
CC = NH * HD    # qkv columns per core (1024)
DS = D // P     # contraction subtiles (16)
ST = S // 512   # 512-wide query blocks (4)
SB = S // P     # 128-row blocks (16)
KO = CC // P    # out-proj contraction subtiles (8)

F32 = mybir.dt.float32
F32R = mybir.dt.float32r
SOFTMAX_SCALE = 1.0 / float(np.sqrt(HD))


def build_nc():
    nc = bacc.Bacc()

    xT_d = nc.dram_tensor("xT", [D, S], F32R, kind="ExternalInput")
    wq_d = nc.dram_tensor("wq", [D, CC], F32R, kind="ExternalInput")
    wk_d = nc.dram_tensor("wk", [D, CC], F32R, kind="ExternalInput")
    wv_d = nc.dram_tensor("wv", [D, CC], F32R, kind="ExternalInput")
    wo_d = nc.dram_tensor("wo", [CC, D], F32R, kind="ExternalInput")
    cosT_d = nc.dram_tensor("cosT", [P, S], F32, kind="ExternalInput")
    sinT_d = nc.dram_tensor("sinT", [P, S], F32, kind="ExternalInput")
    masks_d = nc.dram_tensor("masks", [P, 4 * 512], F32R, kind="ExternalInput")
    onesm_d = nc.dram_tensor("ones_mat", [P, P], F32R, kind="ExternalInput")

    qt_d = nc.dram_tensor("qt", [CC, S], F32R)
    kt_d = nc.dram_tensor("kt", [CC, S], F32R)
    v_d = nc.dram_tensor("v", [S, CC], F32R)
    ot_d = nc.dram_tensor("ot", [CC, S], F32R)

    out_d = nc.dram_tensor("out", [S, D], F32, kind="ExternalOutput")

    lowp = nc.allow_low_precision(reason="float32r rounding of matmul inputs")
    lowp.__enter__()
    with tile.TileContext(nc) as tc:
        with tc.tile_pool(name="xt", bufs=1) as xt_pool:
            XT = xt_pool.tile([P, DS, S], F32R)
            nc.sync.dma_start(XT, xT_d[:].rearrange("(ko p) s -> p ko s", p=P))

            # ---- Phase A1: Q^T, K^T per head with fused RoPE ----
            with (
                tc.tile_pool(name="aqk", bufs=1) as pa,
                tc.tile_pool(name="psA", bufs=1, space="PSUM") as psA,
            ):
                cosT = pa.tile([P, S], F32)
                sinT = pa.tile([P, S], F32)
                nc.sync.dma_start(cosT, cosT_d[:])
                nc.sync.dma_start(sinT, sinT_d[:])
                for h in range(NH):
                    for w_d_, dst_d in ((wq_d, qt_d), (wk_d, kt_d)):
                        wt = pa.tile([P, DS, P], F32R, tag="w", bufs=2)
                        nc.sync.dma_start(
                            wt,
                            w_d_[:, h * P : (h + 1) * P].rearrange(
                                "(ko p) c -> p ko c", p=P
                            ),
                        )
                        ps = [
                            psA.tile([P, 512], F32, tag="proj", bufs=8, name=f"proj{t}")
                            for t in range(ST)
                        ]
                        for ks in range(DS):
                            for t in range(ST):
                                nc.tensor.matmul(
                                    ps[t],
                                    wt[:, ks],
                                    XT[:, ks, t * 512 : (t + 1) * 512],
                                    start=(ks == 0),
                                    stop=(ks == DS - 1),
                                )
                        for t in range(ST):
                            sl = slice(t * 512, (t + 1) * 512)
                            rot = pa.tile([P, 512], F32, tag="rot", bufs=3)
                            ev = pa.tile([P, 512], F32R, tag="ev", bufs=3)
                            nc.scalar.mul(rot[0:64], ps[t][64:128], -1.0)
                            nc.scalar.copy(rot[64:128], ps[t][0:64])
                            nc.vector.tensor_mul(rot, rot, sinT[:, sl])
                            nc.vector.tensor_mul(ev, ps[t], cosT[:, sl])
                            nc.vector.tensor_add(ev, ev, rot)
                            nc.sync.dma_start(dst_d[h * P : (h + 1) * P, sl], ev)

            # ---- Phase A2: V in natural [S, c] layout ----
            with (
                tc.tile_pool(name="av", bufs=1) as pv,
                tc.tile_pool(name="psV", bufs=1, space="PSUM") as psV,
            ):
                for nh in range(2):
                    wvh = []
                    for kh in range(2):
                        wvt = pv.tile(
                            [P, DS // 2, 512], F32R, tag="wv", bufs=3, name=f"wv{kh}"
                        )
                        nc.sync.dma_start(
                            wvt,
                            wv_d[:, nh * 512 : (nh + 1) * 512].rearrange(
                                "(ko p) c -> p ko c", p=P
                            )[:, kh * (DS // 2) : (kh + 1) * (DS // 2), :],
                        )
                        wvh.append(wvt)
                    for sb in range(SB):
                        ps = psV.tile([P, 512], F32, tag="vp", bufs=4)
                        for ks in range(DS):
                            nc.tensor.matmul(
                                ps,
                                XT[:, ks, sb * P : (sb + 1) * P],
                                wvh[ks // (DS // 2)][:, ks % (DS // 2)],
                                start=(ks == 0),
                                stop=(ks == DS - 1),
                            )
                        vev = pv.tile([P, 512], F32R, tag="vev", bufs=4)
                        nc.vector.tensor_copy(vev, ps)
                        nc.sync.dma_start(
                            v_d[sb * P : (sb + 1) * P, nh * 512 : (nh + 1) * 512], vev
                        )

        # ---- Phase B: attention per head ----
        with (
            tc.tile_pool(name="attn", bufs=1) as pb,
            tc.tile_pool(name="psB", bufs=1, space="PSUM") as psB,
        ):
            msk = pb.tile([P, 4, 512], F32R)
            nc.sync.dma_start(msk, masks_d[:].rearrange("p (m f) -> p m f", m=4))
            ones_mat = pb.tile([P, P], F32R)
            nc.sync.dma_start(ones_mat, onesm_d[:])
            for h in range(NH):
                hsl = slice(h * P, (h + 1) * P)
                # quarter-granularity loads so the first S matmul starts
                # after ~1/4 of the head's data has landed
                QTq, KTq, Vq = [], [], []
                for q in range(ST):
                    qt_t = pb.tile([P, 512], F32R, tag=f"qt{q}", bufs=2, name=f"qt{q}")
                    kt_t = pb.tile([P, 512], F32R, tag=f"kt{q}", bufs=2, name=f"kt{q}")
                    v_t = pb.tile([P, 4, P], F32R, tag=f"v{q}", bufs=2, name=f"v{q}")
                    qsl = slice(q * 512, (q + 1) * 512)
                    nc.sync.dma_start(qt_t, qt_d[hsl, qsl])
                    nc.sync.dma_start(kt_t, kt_d[hsl, qsl])
                    nc.sync.dma_start(
                        v_t,
                        v_d[:].rearrange("(jo p) c -> p jo c", p=P)[
                            :, 4 * q : 4 * q + 4, hsl
                        ],
                    )
                    QTq.append(qt_t)
                    KTq.append(kt_t)
                    Vq.append(v_t)
                for t in range(ST):
                    last = 4 * t + 3
                    o_ps = psB.tile([P, 512], F32, tag="o", bufs=2, name="o")
                    l_ps = psB.tile([P, 512], F32, tag="l", bufs=2, name="l")
                    for jj in range(last + 1):
                        s_ps = psB.tile([P, 512], F32, tag="s", bufs=3)
                        nc.tensor.matmul(
                            s_ps,
                            KTq[jj // 4][:, (jj % 4) * P : (jj % 4 + 1) * P],
                            QTq[t],
                            start=True,
                            stop=True,
                        )
                        ex = pb.tile([P, 512], F32R, tag="ex", bufs=6)
                        nc.scalar.activation(
                            ex,
                            s_ps,
                            mybir.ActivationFunctionType.Exp,
                            scale=SOFTMAX_SCALE,
                        )
                        if jj >= 4 * t:
                            nc.vector.tensor_mul(ex, ex, msk[:, jj - 4 * t])
                        nc.tensor.matmul(
                            o_ps,
                            Vq[jj // 4][:, jj % 4],
                            ex,
                            start=(jj == 0),
                            stop=(jj == last),
                        )
                        nc.tensor.matmul(
                            l_ps, ones_mat, ex, start=(jj == 0), stop=(jj == last)
                        )
                    # l_ps rows are all identical (all-ones lhsT); ~18-bit
                    # reciprocal is plenty for softmax normalization
                    li_sb = pb.tile([P, 512], F32, tag="lisb", bufs=2)
                    nc.vector.reciprocal_approx_fast(li_sb, l_ps)
                    oev = pb.tile([P, 512], F32R, tag="oev", bufs=3)
                    nc.vector.tensor_mul(oev, o_ps, li_sb)
                    nc.sync.dma_start(ot_d[hsl, t * 512 : (t + 1) * 512], oev)

        # ---- Phase C: output projection ----
        with (
            tc.tile_pool(name="oproj", bufs=1) as pc,
            tc.tile_pool(name="psC", bufs=1, space="PSUM") as psC,
        ):
            WO = pc.tile([P, KO, D], F32R)
            nc.sync.dma_start(WO, wo_d[:].rearrange("(ko p) n -> p ko n", p=P))
            for sb in range(SB):
                ott = pc.tile([P, KO, P], F32R, tag="ot", bufs=3)
                nc.sync.dma_start(
                    ott,
                    ot_d[:].rearrange("(ko p) s -> p ko s", p=P)[
                        :, :, sb * P : (sb + 1) * P
                    ],
                )
                for t in range(ST):
                    ps = psC.tile([P, 512], F32, tag="op", bufs=4)
                    for ko in range(KO):
                        nc.tensor.matmul(
                            ps,
                            ott[:, ko],
                            WO[:, ko, t * 512 : (t + 1) * 512],
                            start=(ko == 0),
                            stop=(ko == KO - 1),
                        )
                    cev = pc.tile([P, 512], F32, tag="cev", bufs=3)
                    nc.vector.tensor_copy(cev, ps)
                    nc.sync.dma_start(
                        out_d[sb * P : (sb + 1) * P, t * 512 : (t + 1) * 512], cev
                    )

    lowp.__exit__(None, None, None)
    nc.finalize()
    return nc


_NC = None


def _get_nc():
    global _NC
    if _NC is None:
        _NC = build_nc()
    return _NC


def make_in_maps(x, wq, wk, wv, wo, cos, sin):
    """Build the 8 per-core input maps (host-side sharding/layout)."""
    cosT = np.ascontiguousarray(
        np.concatenate([cos.T, cos.T], axis=0), dtype=np.float32
    )  # [128, S]
    sinT = np.ascontiguousarray(
        np.concatenate([sin.T, sin.T], axis=0), dtype=np.float32
    )
    # causal masks for the 4 diagonal j-positions inside a 512-wide query
    # block: keep key (128*m + p) <= query f
    pidx = np.arange(P)[:, None]
    fidx = np.arange(512)[None, :]
    masks = np.concatenate(
        [(fidx >= 128 * m + pidx).astype(np.float32) for m in range(4)], axis=1
    )  # [128, 2048]
    ones_mat = np.ones((P, P), dtype=np.float32)

    in_maps = []
    for c in range(8):
        b, g = divmod(c, 2)
        csl = slice(g * CC, (g + 1) * CC)
        in_maps.append(
            {
                "xT": np.ascontiguousarray(x[b].T),
                "wq": np.ascontiguousarray(wq[:, csl]),
                "wk": np.ascontiguousarray(wk[:, csl]),
                "wv": np.ascontiguousarray(wv[:, csl]),
                "wo": np.ascontiguousarray(wo[csl, :]),
                "cosT": cosT,
                "sinT": sinT,
                "masks": masks,
                "ones_mat": ones_mat,
            }
        )
    return in_maps


def combine(results):
    out = np.empty((B, S, D), dtype=np.float32)
    for b in range(B):
        out[b] = results[2 * b]["out"] + results[2 * b + 1]["out"]
    return out


def kernel(x, wq, wk, wv, wo, cos, sin):
    from concourse.bass_utils import run_bass_kernel_spmd

    nc = _get_nc()
    in_maps = make_in_maps(
        np.asarray(x, dtype=np.float32),
        np.asarray(wq, dtype=np.float32),
        np.asarray(wk, dtype=np.float32),
        np.asarray(wv, dtype=np.float32),
        np.asarray(wo, dtype=np.float32),
        np.asarray(cos, dtype=np.float32),
        np.asarray(sin, dtype=np.float32),
    )
    res = run_bass_kernel_spmd(nc, in_maps, core_ids=list(range(8)))
    return combine(res.results)
